# revision 4
# baseline (speedup 1.0000x reference)
"""Trainium2 Bass kernel for EnhancedOFTQKVLayer.

Computes out[b,s,o] = x[b,s,:] @ filt[o,:]^T + bias[o] where
filt = [Wq @ BD(cayley(q_R)); Wk @ BD(cayley(k_R)); Wv @ BD(cayley(v_R))]
(BD = block-diagonal, cayley(A) = (I-S) inv(I+S+eps I), S = 0.5(A-A^T)).

Distribution: data-parallel - batch b (8 rows) sharded one per NeuronCore;
attn_weight / bias / rotation blocks replicated.

Per-core schedule (v2 - early-start GEMM):
  1. Cayley via SPD Newton-Schulz on P = (1+eps)^2 I - S^2, processed in
     three 2-set waves ordered q -> k -> v so the q-projection rotations
     finish first (~15us) and the main GEMM starts immediately on og0/og1.
  2. x^T produced with ZERO compute-engine cost: x rows cast fp32->bf16 on
     DVE/ScalarE, staged to a bf16 DRAM scratch, then pulled back into SBUF
     pre-transposed by 8 big DMA-xbar (dma transpose) loads of 1 MB each.
  3. W^T for og0/og1 via PE fp32 transposes (fast start); W^T for og2-5 via
     SWDGE cast-DMA to a bf16 DRAM scratch + 4 DMA-xbar transposed loads
     (zero compute cost, soft deadline).
  4. Main matmul in bf16 (fp32 PSUM), three phases: (1) og0/og1 wave over
     the first 8 row-tiles while Newton finishes k/v in PE bubbles,
     (2) row-tile-outer loop over the remaining 24 tiles x all 6 og groups,
     (3) og2-5 for the first 8 row-tiles (pure GEMM tail). Fused bias add
     on DVE per og-pair, one 512 KB output DMA per (row-tile, og-pair).
"""

import numpy as np

import concourse.bass as bass
import concourse.mybir as mybir
import concourse.tile as tile
from concourse import bacc
from concourse.bass import ts
from concourse.masks import make_identity
from concourse.bass_utils import run_bass_kernel_spmd

F32 = mybir.dt.float32
F16 = mybir.dt.float16
BF16 = mybir.dt.bfloat16

MAIN_DT = BF16           # dtype of the big matmul inputs (x, filtT)

HIDDEN = 1024
OUT_DIM = 3 * HIDDEN
SEQ = 4096
P = 128
NBLK = 8                 # 128-blocks per hidden
NROT = 24                # 3 * NBLK rotation blocks
EPS = 1e-6
N_CORES = 8

NSETS = 6                # Newton processes blocks in sets of 4
SETB = 4

# Newton-Schulz schedule (validated offline against the jax reference).
NEWTON_F16 = 7
NEWTON_F32 = 1
SYM_ITERS = {3, 5}       # symmetrize on these fp16 iterations
X0_A = 0.0152174         # X0 = aI + bP (degree-1 minimax init on [1, 260])
X0_B = -5.78922e-05

M_TILES = SEQ // P       # 32 row tiles of 128
SG = SEQ // 512          # 8 row groups of 512 (4 row tiles each)
O_TILES = OUT_DIM // 512  # 6
N1 = 8                   # phase-1 row tiles (og0/og1 early wave)


def build_body(ctx, tc):
    nc = tc.nc

    x = nc.dram_tensor("x", [SEQ, HIDDEN], F32, kind="ExternalInput").ap()
    w = nc.dram_tensor("w", [OUT_DIM, HIDDEN], F32, kind="ExternalInput").ap()
    bias = nc.dram_tensor("bias", [OUT_DIM], F32, kind="ExternalInput").ap()
    rmat = nc.dram_tensor("rmat", [NROT, P, P], F32, kind="ExternalInput").ap()
    out = nc.dram_tensor("out", [SEQ, OUT_DIM], F32, kind="ExternalOutput").ap()

    sub = nc.vector.tensor_sub
    add = nc.vector.tensor_add
    smul = nc.vector.tensor_scalar_mul
    cp = nc.vector.tensor_copy
    scp = nc.scalar.copy

    def bc(t):  # broadcast a [P, P] constant over a set's middle dim
        return t[:].unsqueeze(1).to_broadcast([P, SETB, P])

    # ---- persistent pools ----
    const = ctx.enter_context(tc.tile_pool(name="const", bufs=1))
    ftp = ctx.enter_context(tc.tile_pool(name="ftp", bufs=1))
    xtp = ctx.enter_context(tc.tile_pool(name="xtp", bufs=1))
    qpool = ctx.enter_context(tc.tile_pool(name="qpool", bufs=1))
    dram = ctx.enter_context(tc.tile_pool(name="dram", bufs=1, space="DRAM"))

    ident32 = const.tile([P, P], F32)
    make_identity(nc, ident32)
    eI2 = const.tile([P, P], F32)       # (1+eps)^2 I
    smul(eI2[:], ident32[:], float((1.0 + EPS) ** 2))
    eI12 = const.tile([P, P], F32)      # ((1+eps) + (1+eps)^2) I
    smul(eI12[:], ident32[:], float((1.0 + EPS) + (1.0 + EPS) ** 2))
    twoI = const.tile([P, P], F32)      # 2 I
    smul(twoI[:], ident32[:], 2.0)
    aI0 = const.tile([P, P], F16)       # X0_A * I  (Newton init)
    smul(aI0[:], ident32[:], float(X0_A))

    # bias broadcast (gpsimd queue: this, then only the W og2-5 cast DMAs)
    bias_bc = const.tile([P, OUT_DIM], MAIN_DT)
    with tc.tile_pool(name="biasld", bufs=1) as bl:
        brow = bl.tile([1, OUT_DIM], F32)
        nc.sync.dma_start(brow[:], bias.unsqueeze(0))
        cp(bias_bc[:1, :], brow[:])
    nc.gpsimd.partition_broadcast(bias_bc[:], bias_bc[:1, :])

    # bf16 DRAM scratch for the DMA-xbar transpose paths
    xb = dram.tile([SEQ, HIDDEN], MAIN_DT)       # x cast to bf16
    wb = dram.tile([2048, HIDDEN], MAIN_DT)      # W rows 1024..3071 (og2-5)

    # filtT chunks: ft[k][og][c, o'] = filtT[k*128+c, og*512+o']
    ft = [[ftp.tile([P, 512], MAIN_DT, tag=f"ft{k}_{og}", name=f"ft{k}_{og}")
           for og in range(O_TILES)] for k in range(NBLK)]
    # xt[sg][c, k, s'] = x[sg*512+s', k*128+c]  (x^T, bf16)
    xt = [xtp.tile([P, NBLK, 512], MAIN_DT, tag=f"xt{sg}", name=f"xt{sg}")
          for sg in range(SG)]

    with (
        tc.tile_pool(name="nper", bufs=1) as nper,     # per-wave-slot tiles
        tc.tile_pool(name="ap", bufs=10) as apool,     # rmat block ring
        tc.tile_pool(name="nx", bufs=1) as nxp,        # per-slot iterates
        tc.tile_pool(name="nu", bufs=3) as nup,        # U temp
        tc.tile_pool(name="misc", bufs=1) as misc,
        tc.tile_pool(name="wld", bufs=2) as wld,       # W fp32 rows (og0/og1)
        tc.tile_pool(name="wtsp", bufs=2) as wtsp,     # W^T og ring
        tc.tile_pool(name="xld", bufs=3) as xldp,      # x fp32 rows
        tc.tile_pool(name="xcp", bufs=2) as xcp,       # x bf16 rows
        tc.tile_pool(name="obp", bufs=2) as obp,       # out staging
        tc.tile_pool(name="ps_g", bufs=4, space="PSUM") as ps_g,
        tc.tile_pool(name="ps_out", bufs=4, space="PSUM") as ps_out,
    ):
        # ---- rmat: 24 [128,128] loads through a 12-deep ring, q first ----
        asets = []
        for n in range(NROT):
            a = apool.tile([P, P], F32, tag="a", name=f"a{n}")
            nc.sync.dma_start(a[:], rmat[n])
            asets.append(a)

        # ---- W^T og0/og1: fp32 row loads + PE transposes (fast start) ----
        wts = {}

        def emit_wT_pe(og):
            wts[og] = wtsp.tile([P, NBLK, 512], MAIN_DT, tag="wts",
                                name=f"wts{og}")
            for j4 in range(4):
                ot = og * 4 + j4
                wrow = wld.tile([P, HIDDEN], F32, tag="wrow", name=f"wrow{ot}")
                nc.sync.dma_start(wrow[:], w[ts(ot, P), :])
                for kh in range(2):
                    tpg = ps_g.tile([P, SETB, P], F32, tag="g")
                    for k4 in range(SETB):
                        k = kh * SETB + k4
                        nc.tensor.transpose(tpg[:, k4, :],
                                            wrow[:, ts(k, P)], ident32[:])
                    dst = wts[og][:, ts(kh, SETB), ts(j4, P)]
                    if (j4 + kh) % 2 == 0:
                        cp(dst, tpg[:])
                    else:
                        scp(dst, tpg[:])

        # ---- W^T og2-5: SWDGE cast to bf16 DRAM + DMA-xbar load ----
        def emit_w_cast(og):
            nc.gpsimd.dma_start(wb[ts(og - 2, 512), :], w[ts(og, 512), :])

        def emit_wT_xbar(og):
            wts[og] = wtsp.tile([P, NBLK, 512], MAIN_DT, tag="wts",
                                name=f"wts{og}")
            nc.sync.dma_start(wts[og][:], wb[ts(og - 2, 512), :],
                              transpose=True)

        # ---- x pipeline: load fp32 -> cast bf16 -> store -> xbar load ----
        def emit_x_stage(mt):
            xr = xldp.tile([P, HIDDEN], F32, tag="xr", name=f"xr{mt}")
            nc.sync.dma_start(xr[:], x[ts(mt, P), :])
            xc = xcp.tile([P, HIDDEN], MAIN_DT, tag="xc", name=f"xc{mt}")
            if mt % 2 == 0:
                cp(xc[:], xr[:])
            else:
                scp(xc[:], xr[:])
            nc.scalar.dma_start(xb[ts(mt, P), :], xc[:])

        def emit_xt(sg):
            nc.sync.dma_start(xt[sg][:], xb[ts(sg, 512), :], transpose=True)

        # ---- Newton-Cayley: one wave = two sets of 4 blocks ----
        q_sets = [None] * NSETS

        def newton_wave(s0, s1):
            sets = (s0, s1)
            ss, p32s, p16s, xs = {}, {}, {}, {}
            for s in sets:
                j = s % 2
                tpg = ps_g.tile([P, SETB, P], F32, tag="g")
                for jj in range(SETB):
                    nc.tensor.transpose(tpg[:, jj, :],
                                        asets[s * SETB + jj][:], ident32[:])
                sset = nper.tile([P, SETB, P], F32, tag=f"s{j}", name=f"s{s}")
                for jj in range(SETB):
                    sub(sset[:, jj, :], asets[s * SETB + jj][:], tpg[:, jj, :])
                smul(sset[:], sset[:], 0.5)                  # S
                ss[s] = sset
            for s in sets:
                g = ps_g.tile([P, SETB, P], F32, tag="g")
                for jj in range(SETB):                       # S^T @ S = -S^2
                    nc.tensor.matmul(g[:, jj, :], lhsT=ss[s][:, jj, :],
                                     rhs=ss[s][:, jj, :], start=True,
                                     stop=True)
                j = s % 2
                p32 = nper.tile([P, SETB, P], F32, tag=f"p32{j}",
                                name=f"p32_{s}")
                add(p32[:], bc(eI2), g[:])                   # P=(1+e)^2 I-S^2
                p16 = nper.tile([P, SETB, P], F16, tag=f"p16{j}",
                                name=f"p16_{s}")
                scp(p16[:], p32[:])
                xset = nxp.tile([P, SETB, P], F16, tag=f"x{j}",
                                name=f"x{s}_init")
                smul(xset[:], p32[:], float(X0_B))           # X0 = aI + bP
                add(xset[:], xset[:], bc(aI0))
                # fold B^T = eI12 + (2+e)S - P into the S tile now
                nc.vector.tensor_scalar(ss[s][:], ss[s][:], float(2.0 + EPS),
                                        None, mybir.AluOpType.mult)
                add(ss[s][:], ss[s][:], bc(eI12))
                sub(ss[s][:], ss[s][:], p32[:])
                p32s[s], p16s[s], xs[s] = p32, p16, xset

            for i in range(NEWTON_F16):
                do_sym = i in SYM_ITERS
                for s in sets:
                    j = s % 2
                    g1 = ps_g.tile([P, SETB, P], F32, tag="g")
                    for jj in range(SETB):
                        nc.tensor.matmul(g1[:, jj, :], lhsT=p16s[s][:, jj, :],
                                         rhs=xs[s][:, jj, :], start=True,
                                         stop=True)
                    u = nup.tile([P, SETB, P], F16, tag="u")
                    sub(u[:], bc(twoI), g1[:])               # U = 2I - P X
                    g2 = ps_g.tile([P, SETB, P], F32, tag="g")
                    for jj in range(SETB):                   # X' = X U
                        nc.tensor.matmul(g2[:, jj, :], lhsT=xs[s][:, jj, :],
                                         rhs=u[:, jj, :], start=True,
                                         stop=True)
                    xset = nxp.tile([P, SETB, P], F16, tag=f"x{j}",
                                    name=f"x{s}_{i}")
                    if not do_sym:
                        if j == 0:
                            cp(xset[:], g2[:])               # DVE
                        else:
                            scp(xset[:], g2[:])              # ScalarE
                    else:
                        xcs = misc.tile([P, SETB, P], F32, tag=f"xcs{j}")
                        cp(xcs[:], g2[:])
                        tpg = ps_g.tile([P, SETB, P], F32, tag="g")
                        for jj in range(SETB):
                            nc.tensor.transpose(tpg[:, jj, :], xcs[:, jj, :],
                                                ident32[:])
                        add(xcs[:], xcs[:], tpg[:])
                        nc.scalar.activation(
                            xset[:], xcs[:],
                            mybir.ActivationFunctionType.Copy, scale=0.5)
                    xs[s] = xset

            xfs = {}
            for s in sets:
                j = s % 2
                xf = nxp.tile([P, SETB, P], F32, tag=f"xf{j}",
                              name=f"xf{s}_init")
                if j == 0:
                    cp(xf[:], xs[s][:])
                else:
                    scp(xf[:], xs[s][:])
                xfs[s] = xf
            for i in range(NEWTON_F32):
                g1s = {}
                for s in sets:
                    g1 = ps_g.tile([P, SETB, P], F32, tag="g")
                    for jj in range(SETB):
                        nc.tensor.matmul(g1[:, jj, :], lhsT=p32s[s][:, jj, :],
                                         rhs=xfs[s][:, jj, :], start=True,
                                         stop=True)
                    g1s[s] = g1
                for s in sets:
                    j = s % 2
                    uf = misc.tile([P, SETB, P], F32, tag=f"uf{j}")
                    sub(uf[:], bc(twoI), g1s[s][:])
                    g2 = ps_g.tile([P, SETB, P], F32, tag="g")
                    for jj in range(SETB):
                        nc.tensor.matmul(g2[:, jj, :], lhsT=xfs[s][:, jj, :],
                                         rhs=uf[:, jj, :], start=True,
                                         stop=True)
                    xf = nxp.tile([P, SETB, P], F32, tag=f"xf{j}",
                                  name=f"xf{s}_{i}")
                    if j == 0:
                        cp(xf[:], g2[:])
                    else:
                        scp(xf[:], g2[:])
                    xfs[s] = xf

            # Q = B @ X with B^T = eI12 + (2+e)S - P (pre-folded into ss)
            for s in sets:
                j = s % 2
                g = ps_g.tile([P, SETB, P], F32, tag="g")
                for jj in range(SETB):
                    nc.tensor.matmul(g[:, jj, :], lhsT=ss[s][:, jj, :],
                                     rhs=xfs[s][:, jj, :], start=True,
                                     stop=True)
                qset = qpool.tile([P, SETB, P], MAIN_DT, tag=f"q{s}",
                                  name=f"q{s}")
                if j == 0:
                    cp(qset[:], g[:])
                else:
                    scp(qset[:], g[:])
                q_sets[s] = qset

        def q_lhsT(n):
            return q_sets[n // SETB][:, n % SETB, :]

        # ---- filtT chunks: ft[k][og] = Q^T W^T ----
        def emit_ft(og):
            part = og // 2             # q/k/v
            for k in range(NBLK):
                fg = ps_out.tile([P, 512], F32, tag="po", name=f"fg{og}_{k}")
                nc.tensor.matmul(fg[:], lhsT=q_lhsT(part * NBLK + k),
                                 rhs=wts[og][:, k, :], start=True, stop=True)
                if k % 2 == 0:
                    cp(ft[k][og][:], fg[:])
                else:
                    scp(ft[k][og][:], fg[:])

        # ---- main GEMM: one (row-tile, og-pair) burst = 16 MMs + evict ----
        def emit_gemm_pair(mt, pair):
            sg, sb = mt // 4, mt % 4
            ob = obp.tile([P, 1024], F32, tag="ob", name=f"ob{mt}_{pair}")
            for h in range(2):
                og = pair * 2 + h
                po = ps_out.tile([P, 512], F32, tag="po", name=f"po{mt}_{og}")
                for k in range(NBLK):
                    nc.tensor.matmul(po[:], lhsT=xt[sg][:, k, ts(sb, P)],
                                     rhs=ft[k][og][:],
                                     start=(k == 0), stop=(k == NBLK - 1))
                add(ob[:, ts(h, 512)], po[:], bias_bc[:, ts(og, 512)])
            nc.scalar.dma_start(out[ts(mt, P), ts(pair, 1024)], ob[:])

        # ================= emission order (= priority) =================
        # 1. W og0/og1 path + q-wave Newton + ft og0/og1 (GEMM prereqs)
        emit_wT_pe(0)
        emit_wT_pe(1)
        newton_wave(0, 1)
        emit_ft(0)
        emit_ft(1)

        # 2. x pipeline (DMA-heavy, starts immediately in background)
        for sg in range(SG):
            for mt in range(sg * 4, sg * 4 + 4):
                emit_x_stage(mt)
            emit_xt(sg)

        # 3. W og2-5 casts (SWDGE queue, soft deadlines)
        for og in (2, 3, 4, 5):
            emit_w_cast(og)

        # 4. k/v Newton waves + their W^T/ft (fill phase-1 PE bubbles)
        newton_wave(2, 3)
        emit_wT_xbar(2)
        emit_ft(2)
        emit_wT_xbar(3)
        emit_ft(3)
        newton_wave(4, 5)
        emit_wT_xbar(4)
        emit_ft(4)
        emit_wT_xbar(5)
        emit_ft(5)

        # 5. GEMM phase 1: og0/og1 over the first N1 row tiles
        for mt in range(N1):
            emit_gemm_pair(mt, 0)
        # 6. GEMM phase 2: remaining row tiles, all og groups
        for mt in range(N1, M_TILES):
            for pair in range(3):
                emit_gemm_pair(mt, pair)
        # 7. GEMM phase 3: og2-5 for the first N1 row tiles
        for mt in range(N1):
            emit_gemm_pair(mt, 1)
            emit_gemm_pair(mt, 2)


def build():
    if "nc" in _CACHE:
        return _CACHE["nc"]
    import contextlib

    nc = bacc.Bacc("TRN2", target_bir_lowering=False, debug=False)
    with tile.TileContext(nc) as tc:
        with contextlib.ExitStack() as ctx:
            build_body(ctx, tc)
    nc.compile()
    _CACHE["nc"] = nc
    return nc


_CACHE = {}


def make_in_maps(attn_weight, bias, x, q_R, k_R, v_R):
    rmat = np.ascontiguousarray(
        np.concatenate([q_R, k_R, v_R], axis=0), dtype=np.float32)
    w = np.ascontiguousarray(attn_weight, dtype=np.float32)
    b = np.ascontiguousarray(bias, dtype=np.float32)
    return [
        {"x": np.ascontiguousarray(x[c], dtype=np.float32),
         "w": w, "bias": b, "rmat": rmat}
        for c in range(N_CORES)
    ]


def kernel(attn_weight, bias, x, q_R, k_R, v_R, **run_kwargs):
    nc = build()
    in_maps = make_in_maps(attn_weight, bias, x, q_R, k_R, v_R)
    res = run_bass_kernel_spmd(nc, in_maps, core_ids=list(range(N_CORES)),
                               **run_kwargs)
    out = np.stack([res.results[c]["out"] for c in range(N_CORES)], axis=0)
    _CACHE["last_results"] = res
    return out


# revision 5
# speedup vs baseline: 1.1179x; 1.1179x over previous
"""Trainium2 Bass kernel for EnhancedOFTQKVLayer.

Computes out[b,s,o] = x[b,s,:] @ filt[o,:]^T + bias[o] where
filt = [Wq @ BD(cayley(q_R)); Wk @ BD(cayley(k_R)); Wv @ BD(cayley(v_R))]
(BD = block-diagonal, cayley(A) = (I-S) inv(I+S+eps I), S = 0.5(A-A^T)).

Distribution: data-parallel - batch b (8 rows) sharded one per NeuronCore;
attn_weight / bias / rotation blocks replicated.

Per-core schedule (v3):
  1. ALL transposes are done by DMA, not compute engines: x and W are cast
     fp32->bf16 straight in DRAM by SWDGE cast-DMAs (gpsimd queue), then
     pulled into SBUF pre-transposed by big DMA-xbar (dma transpose) loads
     (one 1 MB load per 512-row group).  Zero PE/DVE/ScalarE cost.
  2. Cayley via SPD Newton-Schulz on P = (1+eps)^2 I - S^2 (iterates are
     polynomials in S^2, hence symmetric -> lhsT=operand works without
     transposes; periodic symmetrization kills roundoff-asymmetry growth).
     All 6 sets of 4 blocks interleaved for chain-latency hiding; fp16
     iterations + fp32 polish.  rmat is pre-permuted on the host to
     [128, 24, 128] so one contiguous DMA loads all rotation blocks.
  3. Main matmul in bf16 (fp32 PSUM), three phases: (1) og0/og1 over the
     first 8 row tiles as soon as the q-projection filtT chunks exist,
     (2) row-tile-outer over the remaining 24 tiles x all 6 og groups,
     (3) og2-5 for the first 8 row tiles (pure GEMM tail; their x^T tiles
     are re-loaded by two extra xbar DMAs so x^T can live in a 4-slot
     ring).  Fused bias add on DVE, 512 KB output DMAs on the ACT queue.
"""

import numpy as np

import concourse.bass as bass
import concourse.mybir as mybir
import concourse.tile as tile
from concourse import bacc
from concourse.bass import ts
from concourse.masks import make_identity
from concourse.bass_utils import run_bass_kernel_spmd

F32 = mybir.dt.float32
F16 = mybir.dt.float16
BF16 = mybir.dt.bfloat16

MAIN_DT = BF16           # dtype of the big matmul inputs (x, filtT)

HIDDEN = 1024
OUT_DIM = 3 * HIDDEN
SEQ = 4096
P = 128
NBLK = 8                 # 128-blocks per hidden
NROT = 24                # 3 * NBLK rotation blocks
EPS = 1e-6
N_CORES = 8

NSETS = 6                # Newton processes blocks in sets of 4
SETB = 4

# Newton-Schulz schedule (validated offline against the jax reference).
NEWTON_F16 = 7
NEWTON_F32 = 1
SYM_ITERS = {3, 5}       # symmetrize on these fp16 iterations
X0_A = 0.0152174         # X0 = aI + bP (degree-1 minimax init on [1, 260])
X0_B = -5.78922e-05

M_TILES = SEQ // P       # 32 row tiles of 128
SG = SEQ // 512          # 8 row groups of 512 (4 row tiles each)
O_TILES = OUT_DIM // 512  # 6
N1 = 8                   # phase-1 row tiles (og0/og1 early wave)


def build_body(ctx, tc):
    nc = tc.nc

    x = nc.dram_tensor("x", [SEQ, HIDDEN], F32, kind="ExternalInput").ap()
    w = nc.dram_tensor("w", [OUT_DIM, HIDDEN], F32, kind="ExternalInput").ap()
    bias = nc.dram_tensor("bias", [OUT_DIM], F32, kind="ExternalInput").ap()
    # host-side pre-permuted rotations: rmt[p, n, c] = rmat[n, p, c]
    rmt_d = nc.dram_tensor("rmt", [P, NROT, P], F32, kind="ExternalInput").ap()
    out = nc.dram_tensor("out", [SEQ, OUT_DIM], F32, kind="ExternalOutput").ap()

    sub = nc.vector.tensor_sub
    add = nc.vector.tensor_add
    smul = nc.vector.tensor_scalar_mul
    cp = nc.vector.tensor_copy
    scp = nc.scalar.copy

    def bc(t):  # broadcast a [P, P] constant over a set's middle dim
        return t[:].unsqueeze(1).to_broadcast([P, SETB, P])

    # ---- persistent pools ----
    const = ctx.enter_context(tc.tile_pool(name="const", bufs=1))
    ftp = ctx.enter_context(tc.tile_pool(name="ftp", bufs=1))
    qpool = ctx.enter_context(tc.tile_pool(name="qpool", bufs=1))
    dram = ctx.enter_context(tc.tile_pool(name="dram", bufs=1, space="DRAM"))

    ident32 = const.tile([P, P], F32)
    make_identity(nc, ident32)
    eI2 = const.tile([P, P], F32)       # (1+eps)^2 I
    smul(eI2[:], ident32[:], float((1.0 + EPS) ** 2))
    eI12 = const.tile([P, P], F32)      # ((1+eps) + (1+eps)^2) I
    smul(eI12[:], ident32[:], float((1.0 + EPS) + (1.0 + EPS) ** 2))
    twoI = const.tile([P, P], F32)      # 2 I
    smul(twoI[:], ident32[:], 2.0)
    aI0 = const.tile([P, P], F16)       # X0_A * I  (Newton init)
    smul(aI0[:], ident32[:], float(X0_A))

    # bf16 DRAM scratch (filled by SWDGE cast-DMAs)
    xb = dram.tile([SEQ, HIDDEN], MAIN_DT)
    wb = dram.tile([OUT_DIM, HIDDEN], MAIN_DT)

    # gpsimd queue order: W og0/1 casts, x sg0/1 casts, bias broadcast,
    # then the remaining casts (everything downstream is deadline-ordered).
    bias_bc = const.tile([P, OUT_DIM], MAIN_DT)
    with tc.tile_pool(name="biasld", bufs=1) as bl:
        brow = bl.tile([1, OUT_DIM], F32)
        nc.sync.dma_start(brow[:], bias.unsqueeze(0))
        cp(bias_bc[:1, :], brow[:])

    def emit_w_cast(og):
        nc.gpsimd.dma_start(wb[ts(og, 512), :], w[ts(og, 512), :])

    def emit_x_cast(sg):
        nc.gpsimd.dma_start(xb[ts(sg, 512), :], x[ts(sg, 512), :])

    emit_w_cast(0)
    emit_w_cast(1)
    emit_x_cast(0)
    emit_x_cast(1)
    nc.gpsimd.partition_broadcast(bias_bc[:], bias_bc[:1, :])
    for og in range(2, O_TILES):
        emit_w_cast(og)
    for sg in range(2, SG):
        emit_x_cast(sg)

    # filtT chunks: ft[k][og][c, o'] = filtT[k*128+c, og*512+o']
    ft = [[ftp.tile([P, 512], MAIN_DT, tag=f"ft{k}_{og}", name=f"ft{k}_{og}")
           for og in range(O_TILES)] for k in range(NBLK)]

    with (
        tc.tile_pool(name="rmt", bufs=1) as rmtp,
        tc.tile_pool(name="nper", bufs=1) as nper,     # per-set persistents
        tc.tile_pool(name="nx", bufs=1) as nxp,        # per-set iterates
        tc.tile_pool(name="nu", bufs=3) as nup,        # U temp
        tc.tile_pool(name="misc", bufs=1) as misc,
        tc.tile_pool(name="wtsp", bufs=2) as wtsp,     # W^T og ring
        tc.tile_pool(name="xtp", bufs=4) as xtp,       # x^T sg ring
        tc.tile_pool(name="obp", bufs=3) as obp,       # out staging
        tc.tile_pool(name="ps_g", bufs=4, space="PSUM") as ps_g,
        tc.tile_pool(name="ps_out", bufs=4, space="PSUM") as ps_out,
    ):
        # ---- rotations: one contiguous 1.5 MB load ----
        rmt = rmtp.tile([P, NROT, P], F32)
        nc.sync.dma_start(rmt[:], rmt_d)

        def aset(n):
            return rmt[:, n, :]

        # ---- W^T / x^T via DMA-xbar transposed loads ----
        wts = {}

        def emit_wT(og):
            wts[og] = wtsp.tile([P, NBLK, 512], MAIN_DT, tag="wts",
                                name=f"wts{og}")
            nc.sync.dma_start(wts[og][:], wb[ts(og, 512), :], transpose=True)

        xts = {}

        def emit_xt(sg, gen=""):
            t = xtp.tile([P, NBLK, 512], MAIN_DT, tag="xt",
                         name=f"xt{sg}{gen}")
            nc.sync.dma_start(t[:], xb[ts(sg, 512), :], transpose=True)
            xts[sg] = t

        # ---------- Newton-Cayley: 6 interleaved sets of 4 blocks ----------
        s_s, p32_s, p16_s, x_s = [], [], [], []
        for s in range(NSETS):
            tpg = ps_g.tile([P, SETB, P], F32, tag="g")
            for j in range(SETB):
                nc.tensor.transpose(tpg[:, j, :], aset(s * SETB + j),
                                    ident32[:])
            sset = nper.tile([P, SETB, P], F32, tag=f"s{s}", name=f"s{s}")
            for j in range(SETB):
                sub(sset[:, j, :], aset(s * SETB + j), tpg[:, j, :])
            smul(sset[:], sset[:], 0.5)                  # S
            s_s.append(sset)
        g_s = []
        for s in range(NSETS):
            g = ps_g.tile([P, SETB, P], F32, tag="g")
            for j in range(SETB):                        # S^T @ S = -S^2
                nc.tensor.matmul(g[:, j, :], lhsT=s_s[s][:, j, :],
                                 rhs=s_s[s][:, j, :], start=True, stop=True)
            g_s.append(g)
            if s % 2 == 1:        # drain the g ring (only 4 banks)
                for sp in (s - 1, s):
                    p32s = nper.tile([P, SETB, P], F32, tag=f"p32{sp}",
                                     name=f"p32{sp}")
                    add(p32s[:], bc(eI2), g_s[sp][:])    # P = (1+e)^2 I - S^2
                    p16s = nper.tile([P, SETB, P], F16, tag=f"p16{sp}",
                                     name=f"p16{sp}")
                    scp(p16s[:], p32s[:])
                    xset = nxp.tile([P, SETB, P], F16, tag=f"x{sp}",
                                    name=f"x{sp}_init")
                    smul(xset[:], p32s[:], float(X0_B))  # X0 = aI + bP
                    add(xset[:], xset[:], bc(aI0))
                    # fold B^T = eI12 + (2+e)S - P into the S tile now
                    nc.vector.tensor_scalar(s_s[sp][:], s_s[sp][:],
                                            float(2.0 + EPS), None,
                                            mybir.AluOpType.mult)
                    add(s_s[sp][:], s_s[sp][:], bc(eI12))
                    sub(s_s[sp][:], s_s[sp][:], p32s[:])
                    p32_s.append(p32s)
                    p16_s.append(p16s)
                    x_s.append(xset)

        for i in range(NEWTON_F16):
            do_sym = i in SYM_ITERS
            for s in range(NSETS):
                g1 = ps_g.tile([P, SETB, P], F32, tag="g")
                for j in range(SETB):
                    nc.tensor.matmul(g1[:, j, :], lhsT=p16_s[s][:, j, :],
                                     rhs=x_s[s][:, j, :], start=True,
                                     stop=True)
                u = nup.tile([P, SETB, P], F16, tag="u")
                sub(u[:], bc(twoI), g1[:])               # U = 2I - P X (DVE)
                g2 = ps_g.tile([P, SETB, P], F32, tag="g")
                for j in range(SETB):                    # X' = X U
                    nc.tensor.matmul(g2[:, j, :], lhsT=x_s[s][:, j, :],
                                     rhs=u[:, j, :], start=True, stop=True)
                xset = nxp.tile([P, SETB, P], F16, tag=f"x{s}",
                                name=f"x{s}_{i}")
                if not do_sym:
                    if s == 0:
                        cp(xset[:], g2[:])               # DVE
                    else:
                        scp(xset[:], g2[:])              # ScalarE
                else:
                    xc = misc.tile([P, SETB, P], F32, tag="xc")
                    cp(xc[:], g2[:])
                    tpg = ps_g.tile([P, SETB, P], F32, tag="g")
                    for j in range(SETB):
                        nc.tensor.transpose(tpg[:, j, :], xc[:, j, :],
                                            ident32[:])
                    add(xc[:], xc[:], tpg[:])
                    nc.scalar.activation(xset[:], xc[:],
                                         mybir.ActivationFunctionType.Copy,
                                         scale=0.5)
                x_s[s] = xset

        xf_s = []
        for s in range(NSETS):
            xf = nxp.tile([P, SETB, P], F32, tag=f"xf{s}", name=f"xf{s}_init")
            if s % 2 == 0:
                cp(xf[:], x_s[s][:])
            else:
                scp(xf[:], x_s[s][:])
            xf_s.append(xf)
        for i in range(NEWTON_F32):
            for sh in range(NSETS // 2):      # pairs, to respect the g ring
                pair = (2 * sh, 2 * sh + 1)
                g1s = {}
                for s in pair:
                    g1 = ps_g.tile([P, SETB, P], F32, tag="g")
                    for j in range(SETB):
                        nc.tensor.matmul(g1[:, j, :], lhsT=p32_s[s][:, j, :],
                                         rhs=xf_s[s][:, j, :], start=True,
                                         stop=True)
                    g1s[s] = g1
                for s in pair:
                    uf = misc.tile([P, SETB, P], F32, tag="uf")
                    sub(uf[:], bc(twoI), g1s[s][:])
                    g2 = ps_g.tile([P, SETB, P], F32, tag="g")
                    for j in range(SETB):
                        nc.tensor.matmul(g2[:, j, :], lhsT=xf_s[s][:, j, :],
                                         rhs=uf[:, j, :], start=True,
                                         stop=True)
                    xf = nxp.tile([P, SETB, P], F32, tag=f"xf{s}",
                                  name=f"xf{s}_{i}")
                    if s % 2 == 0:
                        cp(xf[:], g2[:])
                    else:
                        scp(xf[:], g2[:])
                    xf_s[s] = xf

        # Q = B @ X with B^T = eI12 + (2+e)S - P (pre-folded into s_s)
        q_s = []
        for s in range(NSETS):
            g = ps_g.tile([P, SETB, P], F32, tag="g")
            for j in range(SETB):
                nc.tensor.matmul(g[:, j, :], lhsT=s_s[s][:, j, :],
                                 rhs=xf_s[s][:, j, :], start=True, stop=True)
            qset = qpool.tile([P, SETB, P], MAIN_DT, tag=f"q{s}", name=f"q{s}")
            if s % 2 == 0:
                cp(qset[:], g[:])
            else:
                scp(qset[:], g[:])
            q_s.append(qset)

        def q_lhsT(n):
            return q_s[n // SETB][:, n % SETB, :]

        # ---- filtT chunks: ft[k][og] = Q^T W^T ----
        def emit_ft(og):
            part = og // 2             # q/k/v
            for k in range(NBLK):
                fg = ps_out.tile([P, 512], F32, tag="po", name=f"fg{og}_{k}")
                nc.tensor.matmul(fg[:], lhsT=q_lhsT(part * NBLK + k),
                                 rhs=wts[og][:, k, :], start=True, stop=True)
                if k % 2 == 0:
                    cp(ft[k][og][:], fg[:])
                else:
                    scp(ft[k][og][:], fg[:])

        # ---- main GEMM bursts ----
        def emit_gemm(mt, og_lo, n_og):
            sg, sb = mt // 4, mt % 4
            ob = obp.tile([P, 512 * n_og], F32, tag="ob",
                          name=f"ob{mt}_{og_lo}")
            for h in range(n_og):
                og = og_lo + h
                po = ps_out.tile([P, 512], F32, tag="po", name=f"po{mt}_{og}")
                for k in range(NBLK):
                    nc.tensor.matmul(po[:], lhsT=xts[sg][:, k, ts(sb, P)],
                                     rhs=ft[k][og][:],
                                     start=(k == 0), stop=(k == NBLK - 1))
                add(ob[:, ts(h, 512)], po[:], bias_bc[:, ts(og, 512)])
            nc.scalar.dma_start(
                out[ts(mt, P), bass.ds(og_lo * 512, 512 * n_og)], ob[:])

        # ================= emission order (= priority) =================
        emit_wT(0)
        emit_xt(0)
        emit_wT(1)
        emit_xt(1)
        emit_ft(0)
        emit_ft(1)
        # phase 1: og0 then og1 over the first N1 row tiles; ft og2-5
        # builds are emitted between chunks so they slot in as soon as
        # their xbar loads land.
        for mt in range(N1):
            emit_gemm(mt, 0, 1)
            if mt == 1:
                emit_wT(2)
            if mt == 3:
                emit_wT(3)
        emit_ft(2)
        emit_xt(2)
        for mt in range(N1):
            emit_gemm(mt, 1, 1)
            if mt == 1:
                emit_wT(4)
            if mt == 3:
                emit_wT(5)
        emit_ft(3)
        emit_ft(4)
        emit_ft(5)
        emit_xt(3)
        # phase 2: remaining row tiles, all og groups
        for mt in range(N1, M_TILES):
            if mt % 4 == 0 and mt + 8 < M_TILES:
                emit_xt((mt + 8) // 4)      # prefetch sg two groups ahead
            if mt == 24:
                emit_xt(0, gen="b")          # phase-3 reloads
            if mt == 28:
                emit_xt(1, gen="b")
            for pair in range(3):
                emit_gemm(mt, 2 * pair, 2)
        # phase 3: og2-5 for the first N1 row tiles
        for mt in range(N1):
            emit_gemm(mt, 2, 2)
            emit_gemm(mt, 4, 2)


def build():
    if "nc" in _CACHE:
        return _CACHE["nc"]
    import contextlib

    nc = bacc.Bacc("TRN2", target_bir_lowering=False, debug=False)
    with tile.TileContext(nc) as tc:
        with contextlib.ExitStack() as ctx:
            build_body(ctx, tc)
    nc.compile()
    _CACHE["nc"] = nc
    return nc


_CACHE = {}


def make_in_maps(attn_weight, bias, x, q_R, k_R, v_R):
    rmat = np.concatenate([q_R, k_R, v_R], axis=0).astype(np.float32)
    rmt = np.ascontiguousarray(rmat.transpose(1, 0, 2))  # [P, NROT, P]
    w = np.ascontiguousarray(attn_weight, dtype=np.float32)
    b = np.ascontiguousarray(bias, dtype=np.float32)
    return [
        {"x": np.ascontiguousarray(x[c], dtype=np.float32),
         "w": w, "bias": b, "rmt": rmt}
        for c in range(N_CORES)
    ]


def kernel(attn_weight, bias, x, q_R, k_R, v_R, **run_kwargs):
    nc = build()
    in_maps = make_in_maps(attn_weight, bias, x, q_R, k_R, v_R)
    res = run_bass_kernel_spmd(nc, in_maps, core_ids=list(range(N_CORES)),
                               **run_kwargs)
    out = np.stack([res.results[c]["out"] for c in range(N_CORES)], axis=0)
    _CACHE["last_results"] = res
    return out
